# revision 47
# baseline (speedup 1.0000x reference)
"""Embedding lookup (weight[input_ids]) on 8 Trainium2 NeuronCores.

Strategy: data-parallel over tokens. The 4x2048=8192 token ids are split
into 8 shards of 1024 tokens; every core holds the full [32000, 128] f32
table in HBM and uses the SWDGE dma_gather instruction to pull its 1024
rows (512 B each) directly from HBM into SBUF, then stores the gathered
block to its output shard with fully-contiguous DMAs.

Token->SBUF placement is chosen on the host so the SBUF->HBM store is
contiguous: gather position j handles token t = (j%128)*8 + j//128, which
lands token t's row at SBUF [partition t//8, block t%8].  Partition p then
holds tokens p*8..p*8+7 back to back, so the store AP is a plain
[128, 1024]f32 -> flat DRAM copy and the output shard comes out in natural
token order.

Pipeline (per core), store_mode="scatter" (default):
  SP  : ids DMA (HWDGE: wrapped gather indices + constant identity scatter
        indices in one [128,128]i16 tensor) -> drain -> engine-sem handoff
        to Pool (skips the ~900ns DMA-sem propagation)
  Pool: gpsimd 'mlp' ucode library load (overlaps the ids DMA), then on
        SWDGE queue 0 the gather split into (640, 384) chunks; while those
        transfer, the stores are pre-generated on SWDGE queue 1 as
        dma_scatter_add ops (identity indices onto the zero-initialized
        output - both runtime paths pre-zero/donate-zero ExternalOutputs)
        with prepare_only=True, then fired by trigger_dma the moment each
        gather's completion semaphore lands.  This removes the HWDGE
        store path entirely (no 625ns descriptor-gen + 650ns DGE-DMA delay
        per store) and keeps the SDMA engines almost continuously busy.
Completion is guaranteed by the block-exit engine drains rather than a
final sem wait.  The four framework const-memsets are stripped from the
preamble (nothing reads them), shortening the entry barrier.

TimelineSim (production cost model) estimate: ~7.2us per core; the
remaining time is the serial SWDGE descriptor-generation chain (994ns
fixed per op x 4 ops), the ids-load latency, and DMA transfers at modeled
full bandwidth.
"""

import numpy as np

VOCAB = 32000
EMBED = 128
N_CORES = 8
B, S = 4, 2048
N = B * S                 # 8192 tokens total
NPC = N // N_CORES        # 1024 tokens per core
BLK = NPC // 128          # 8 blocks of 128 gather positions
IDXW = NPC // 16          # 64 idx columns in the wrapped idx layout

DEFAULT_CHUNKS = (640, 384)

_NC_CACHE = {}


def build_nc(chunk_sizes=DEFAULT_CHUNKS, split_store=False, no_gpsimd_drain=False,
             ids_drain_handoff=True, no_store_wait=True,
             strip_const_memsets=True, warmup_gather=False,
             store_mode=None, ids_split=True):
    """Build the per-core Bass program (identical on all 8 cores).

    store_mode:
      "hwdge"   - per-chunk SBUF->HBM DMACopy stores on SP (HWDGE)
      "scatter" - per-chunk dma_scatter_add (identity indices) onto the
                  zero-donated output, pre-generated on SWDGE queue 1 with
                  prepare_only and fired by trigger_dma as soon as the
                  matching gather's semaphore lands.  Skips the HWDGE
                  descriptor-gen + DGE-DMA-delay chain on the store path.
    """
    from contextlib import ExitStack

    import concourse.bacc as bacc
    import concourse.mybir as mybir
    from concourse import library_config

    if store_mode is None:
        store_mode = DEFAULT_STORE_MODE
    chunk_sizes = tuple(chunk_sizes)
    assert sum(chunk_sizes) == NPC
    assert all(c % 128 == 0 for c in chunk_sizes)
    chunks = len(chunk_sizes)
    starts = [sum(chunk_sizes[:i]) for i in range(chunks)]
    scatter = store_mode == "scatter"

    nc = bacc.Bacc("TRN2", target_bir_lowering=False, num_devices=N_CORES,
                   num_swdge_queues=2 if scatter else 1)

    ids_cols = 2 * IDXW if scatter else IDXW
    ids_d = nc.dram_tensor("ids", [128, ids_cols], mybir.dt.int16,
                           kind="ExternalInput")
    w_d = nc.dram_tensor(
        "weight", [VOCAB, EMBED], mybir.dt.float32, kind="ExternalInput"
    )
    out_d = nc.dram_tensor(
        "out", [NPC, EMBED], mybir.dt.float32, kind="ExternalOutput"
    )

    with ExitStack() as stack:
        block = stack.enter_context(nc.Block(no_gpsimd_drain=no_gpsimd_drain))
        ids_sem = stack.enter_context(nc.semaphore("ids_sem"))
        ids_dma_sem = stack.enter_context(nc.semaphore("ids_dma_sem"))
        st_sem = stack.enter_context(nc.semaphore("st_sem"))
        gath_sems = [
            stack.enter_context(nc.semaphore(f"gath_sem{c}")) for c in range(chunks)
        ]
        if scatter:
            prep_sem = stack.enter_context(nc.semaphore("prep_sem"))
            sc_sems = [
                stack.enter_context(nc.semaphore(f"sc_sem{c}"))
                for c in range(chunks)
            ]
            ids_split = ids_split and chunks >= 2
            if ids_split:
                sidx_sem = stack.enter_context(nc.semaphore("sidx_sem"))
                act_dma_sem = stack.enter_context(nc.semaphore("act_dma_sem"))
        else:
            ids_split = False
        idx_t = stack.enter_context(
            nc.sbuf_tensor("idx_t", [128, ids_cols], mybir.dt.int16)
        )
        gath_t = stack.enter_context(
            nc.sbuf_tensor("gath_t", [128, NPC], mybir.dt.float32)
        )
        if warmup_gather:
            wu_sem = stack.enter_context(nc.semaphore("wu_sem"))
            wu_dma_sem = stack.enter_context(nc.semaphore("wu_dma_sem"))
            wu_idx = stack.enter_context(
                nc.sbuf_tensor("wu_idx", [128, 1], mybir.dt.int16)
            )
            wu_out = stack.enter_context(
                nc.sbuf_tensor("wu_out", [128, EMBED], mybir.dt.float32)
            )

        out_v = out_d.ap().rearrange("(p x) e -> p (x e)", p=128)  # [128, NPC]

        @block.gpsimd
        def _(g):
            g.load_library(library_config.mlp)
            # hoist the num_idxs registers so the ids wait attaches to the
            # first gather, not a register move
            regs = {}
            for ch in sorted(set(chunk_sizes)):
                regs[ch] = g.to_reg(ch)
            if warmup_gather:
                # run the gather ucode path once (row 0, 16 idxs) while the
                # ids DMA is in flight -- warms the Q7 icache off the
                # critical path
                g.memset(wu_idx[:], 0).then_inc(wu_sem, 1)
                g.wait_ge(wu_sem, 1)
                g.dma_gather(
                    wu_out[:].rearrange("p (b e) -> p b e", e=EMBED),
                    w_d.ap(),
                    wu_idx[:],
                    16,
                    g.to_reg(16),
                    EMBED,
                ).then_inc(wu_dma_sem, 16)
            g.wait_ge(ids_sem, 16)
            for c, (st, ch) in enumerate(zip(starts, chunk_sizes)):
                if scatter and ids_split and c == 1:
                    # chunk 1's idx columns ride the ACT-side DMA
                    g.wait_ge(sidx_sem, 16)
                g.dma_gather(
                    gath_t[:, st : st + ch].rearrange("p (b e) -> p b e", e=EMBED),
                    w_d.ap(),
                    idx_t[:, st // 16 : (st + ch) // 16],
                    ch,           # num_idxs
                    regs[ch],     # num_idxs_reg (all indices valid)
                    EMBED,        # elem_size (one table row)
                ).then_inc(gath_sems[c], 16)
            if scatter:
                # pre-generate the store descriptors on queue 1 while the
                # gathers transfer; src data is only read at trigger time
                for c, (st, ch) in enumerate(zip(starts, chunk_sizes)):
                    g.dma_scatter_add(
                        out_d.ap(),
                        gath_t[:, st : st + ch].rearrange(
                            "p (b e) -> p b e", e=EMBED
                        ),
                        idx_t[:, IDXW + st // 16 : IDXW + (st + ch) // 16],
                        ch,
                        regs[ch],
                        EMBED,
                        elem_step=EMBED,
                        prepare_only=True,
                        sem=sc_sems[c],
                        queue_num=1,
                    ).then_inc(prep_sem, 1)
                for c in range(chunks):
                    g.wait_ge(prep_sem, c + 1)
                    g.wait_ge(gath_sems[c], 16)
                    g.trigger_dma(1, queue_num=1)

        @block.sync
        def _(sp):
            # SP carries only what gates the first gather; with ids_split the
            # rest (chunk1 idx cols + scatter constants) rides ACT in parallel
            sp_cols = slice(0, chunk_sizes[0] // 16) if (scatter and ids_split) \
                else slice(0, ids_cols)
            if ids_drain_handoff:
                # drain waits for the HWDGE FIFO (data landed), then a cheap
                # engine sem-inc signals Pool -- skips the 900ns DMA sem prop
                sp.dma_start(idx_t[:, sp_cols], ids_d.ap()[:, sp_cols]).then_inc(
                    ids_dma_sem, 16
                )
                sp.drain().then_inc(ids_sem, 16)
            else:
                sp.dma_start(idx_t[:, sp_cols], ids_d.ap()[:, sp_cols]).then_inc(
                    ids_sem, 16
                )
            if scatter:
                return
            for c, (st, ch) in enumerate(zip(starts, chunk_sizes)):
                if split_store and c % 2 == 1:
                    continue  # handled by ACT below
                sp.wait_ge(gath_sems[c], 16)
                sp.dma_start(
                    out_v[:, st : st + ch], gath_t[:, st : st + ch]
                ).then_inc(st_sem, 16)
            if not no_store_wait:
                sp.wait_ge(st_sem, 16 * chunks)

        if scatter and ids_split:
            a_cols = slice(chunk_sizes[0] // 16, ids_cols)

            @block.scalar
            def _(act):
                act.dma_start(
                    idx_t[:, a_cols], ids_d.ap()[:, a_cols]
                ).then_inc(act_dma_sem, 16)
                act.drain().then_inc(sidx_sem, 16)

        if split_store and not scatter:

            @block.scalar
            def _(act):
                for c, (st, ch) in enumerate(zip(starts, chunk_sizes)):
                    if c % 2 == 0:
                        continue
                    act.wait_ge(gath_sems[c], 16)
                    act.dma_start(
                        out_v[:, st : st + ch], gath_t[:, st : st + ch]
                    ).then_inc(st_sem, 16)

    if strip_const_memsets:
        # The framework preamble memsets four const-* SBUF tiles this kernel
        # never reads; dropping them shortens the Pool preamble before the
        # entry barrier.
        import concourse.mybir as mybir

        blk = nc.m.functions[0].blocks[0]
        keep = [
            i
            for i in blk.instructions
            if not (
                isinstance(i, mybir.InstMemset)
                and i.outs
                and str(getattr(i.outs[0], "memref", "")).startswith("const-")
            )
        ]
        blk.instructions = keep

    nc.compile()
    return nc


def _get_nc(store_mode="hwdge"):
    if store_mode not in _NC_CACHE:
        _NC_CACHE[store_mode] = build_nc(store_mode=store_mode)
    return _NC_CACHE[store_mode]


def _wrap16(vals):
    """[NPC] -> [128, NPC//16] int16 in dma_gather's wrapped idx layout:
    value j at partition j%16, column j//16, replicated to all 8 gpsimd
    cores (16 partitions each)."""
    w = vals.reshape(IDXW, 16).T                         # [16, 64]
    return np.tile(w, (8, 1)).astype(np.int16)           # [128, 64]


def prep_ids(ids_flat, store_mode="hwdge"):
    """Per-core wrapped int16 idx arrays.

    hwdge:   gather position j looks up token t(j) = (j%128)*8 + j//128 so
             the SBUF tile stores contiguously (permuted layout).
    scatter: gather position j looks up token j (natural order); a constant
             identity block (scatter row indices) is appended so the
             dma_scatter_add writes out[j].
    """
    per_core = []
    ident = _wrap16(np.arange(NPC, dtype=np.int64))
    for c in range(N_CORES):
        shard = ids_flat[c * NPC : (c + 1) * NPC]
        if store_mode == "scatter":
            full = np.concatenate([_wrap16(shard), ident], axis=1)  # [128, 128]
        else:
            pos = shard.reshape(128, BLK).T.reshape(-1)  # pos[j] = shard[t(j)]
            full = _wrap16(pos)                          # [128, 64]
        per_core.append(np.ascontiguousarray(full))
    return per_core


DEFAULT_STORE_MODE = "scatter"


def run_spmd(inputs, trace=False, nc=None, store_mode=None):
    """Returns (output [4,2048,128] f32, BassKernelResults)."""
    from concourse.bass_utils import run_bass_kernel_spmd

    if store_mode is None:
        store_mode = DEFAULT_STORE_MODE
    ids = np.asarray(inputs["input_ids"]).reshape(-1).astype(np.int64)
    w = np.ascontiguousarray(np.asarray(inputs["weight"], dtype=np.float32))
    assert ids.shape == (N,) and w.shape == (VOCAB, EMBED)

    in_maps = [
        {"ids": ids_c, "weight": w} for ids_c in prep_ids(ids, store_mode)
    ]
    res = run_bass_kernel_spmd(
        nc if nc is not None else _get_nc(store_mode),
        in_maps,
        core_ids=list(range(N_CORES)),
        trace=trace,
    )
    shards = [r["out"] for r in res.results]
    out = np.concatenate(shards, axis=0).reshape(B, S, EMBED)
    return np.ascontiguousarray(out.astype(np.float32)), res


def kernel(**inputs):
    out, _ = run_spmd(inputs, trace=False)
    return out


# revision 50
# speedup vs baseline: 1.0359x; 1.0359x over previous
"""Embedding lookup (weight[input_ids]) on 8 Trainium2 NeuronCores.

Strategy: data-parallel over tokens. The 4x2048=8192 token ids are split
into 8 shards of 1024 tokens; every core holds the full [32000, 128] f32
table in HBM and uses the SWDGE dma_gather instruction to pull its 1024
rows (512 B each) directly from HBM into SBUF, then stores the gathered
block to its output shard with fully-contiguous DMAs.

Token->SBUF placement is chosen on the host so the SBUF->HBM store is
contiguous: gather position j handles token t = (j%128)*8 + j//128, which
lands token t's row at SBUF [partition t//8, block t%8].  Partition p then
holds tokens p*8..p*8+7 back to back, so the store AP is a plain
[128, 1024]f32 -> flat DRAM copy and the output shard comes out in natural
token order.

Pipeline (per core), store_mode="scatter" (default):
  SP  : ids DMA (HWDGE: wrapped gather indices + constant identity scatter
        indices in one [128,128]i16 tensor) -> drain -> engine-sem handoff
        to Pool (skips the ~900ns DMA-sem propagation)
  Pool: gpsimd 'mlp' ucode library load (overlaps the ids DMA), then on
        SWDGE queue 0 the gather split into (640, 384) chunks; while those
        transfer, the stores are pre-generated on SWDGE queue 1 as
        dma_scatter_add ops (identity indices onto the zero-initialized
        output - both runtime paths pre-zero/donate-zero ExternalOutputs)
        with prepare_only=True, then fired by trigger_dma the moment each
        gather's completion semaphore lands.  This removes the HWDGE
        store path entirely (no 625ns descriptor-gen + 650ns DGE-DMA delay
        per store) and keeps the SDMA engines almost continuously busy.
Completion is guaranteed by the block-exit engine drains rather than a
final sem wait.  The four framework const-memsets are stripped from the
preamble (nothing reads them), shortening the entry barrier.

TimelineSim (production cost model) estimate: ~7.2us per core; the
remaining time is the serial SWDGE descriptor-generation chain (994ns
fixed per op x 4 ops), the ids-load latency, and DMA transfers at modeled
full bandwidth.
"""

import numpy as np

VOCAB = 32000
EMBED = 128
N_CORES = 8
B, S = 4, 2048
N = B * S                 # 8192 tokens total
NPC = N // N_CORES        # 1024 tokens per core
BLK = NPC // 128          # 8 blocks of 128 gather positions
IDXW = NPC // 16          # 64 idx columns in the wrapped idx layout

DEFAULT_CHUNKS = (640, 384)

_NC_CACHE = {}


def build_nc(chunk_sizes=DEFAULT_CHUNKS, split_store=False, no_gpsimd_drain=False,
             ids_drain_handoff=True, no_store_wait=True,
             strip_const_memsets=True, warmup_gather=False,
             store_mode=None, ids_split=True, strip_entry_barrier=True):
    """Build the per-core Bass program (identical on all 8 cores).

    store_mode:
      "hwdge"   - per-chunk SBUF->HBM DMACopy stores on SP (HWDGE)
      "scatter" - per-chunk dma_scatter_add (identity indices) onto the
                  zero-donated output, pre-generated on SWDGE queue 1 with
                  prepare_only and fired by trigger_dma as soon as the
                  matching gather's semaphore lands.  Skips the HWDGE
                  descriptor-gen + DGE-DMA-delay chain on the store path.
    """
    from contextlib import ExitStack

    import concourse.bacc as bacc
    import concourse.mybir as mybir
    from concourse import library_config

    if store_mode is None:
        store_mode = DEFAULT_STORE_MODE
    chunk_sizes = tuple(chunk_sizes)
    assert sum(chunk_sizes) == NPC
    assert all(c % 128 == 0 for c in chunk_sizes)
    chunks = len(chunk_sizes)
    starts = [sum(chunk_sizes[:i]) for i in range(chunks)]
    scatter = store_mode == "scatter"

    nc = bacc.Bacc("TRN2", target_bir_lowering=False, num_devices=N_CORES,
                   num_swdge_queues=2 if scatter else 1)

    ids_cols = 2 * IDXW if scatter else IDXW
    ids_d = nc.dram_tensor("ids", [128, ids_cols], mybir.dt.int16,
                           kind="ExternalInput")
    w_d = nc.dram_tensor(
        "weight", [VOCAB, EMBED], mybir.dt.float32, kind="ExternalInput"
    )
    out_d = nc.dram_tensor(
        "out", [NPC, EMBED], mybir.dt.float32, kind="ExternalOutput"
    )

    with ExitStack() as stack:
        block = stack.enter_context(nc.Block(no_gpsimd_drain=no_gpsimd_drain))
        ids_sem = stack.enter_context(nc.semaphore("ids_sem"))
        ids_dma_sem = stack.enter_context(nc.semaphore("ids_dma_sem"))
        st_sem = stack.enter_context(nc.semaphore("st_sem"))
        gath_sems = [
            stack.enter_context(nc.semaphore(f"gath_sem{c}")) for c in range(chunks)
        ]
        if scatter:
            prep_sem = stack.enter_context(nc.semaphore("prep_sem"))
            sc_sems = [
                stack.enter_context(nc.semaphore(f"sc_sem{c}"))
                for c in range(chunks)
            ]
            ids_split = ids_split and chunks >= 2
            if ids_split:
                sidx_sem = stack.enter_context(nc.semaphore("sidx_sem"))
                act_dma_sem = stack.enter_context(nc.semaphore("act_dma_sem"))
        else:
            ids_split = False
        idx_t = stack.enter_context(
            nc.sbuf_tensor("idx_t", [128, ids_cols], mybir.dt.int16)
        )
        gath_t = stack.enter_context(
            nc.sbuf_tensor("gath_t", [128, NPC], mybir.dt.float32)
        )
        if warmup_gather:
            wu_sem = stack.enter_context(nc.semaphore("wu_sem"))
            wu_dma_sem = stack.enter_context(nc.semaphore("wu_dma_sem"))
            wu_idx = stack.enter_context(
                nc.sbuf_tensor("wu_idx", [128, 1], mybir.dt.int16)
            )
            wu_out = stack.enter_context(
                nc.sbuf_tensor("wu_out", [128, EMBED], mybir.dt.float32)
            )

        out_v = out_d.ap().rearrange("(p x) e -> p (x e)", p=128)  # [128, NPC]

        @block.gpsimd
        def _(g):
            g.load_library(library_config.mlp)
            # hoist the num_idxs registers so the ids wait attaches to the
            # first gather, not a register move
            regs = {}
            for ch in sorted(set(chunk_sizes)):
                regs[ch] = g.to_reg(ch)
            if warmup_gather:
                # run the gather ucode path once (row 0, 16 idxs) while the
                # ids DMA is in flight -- warms the Q7 icache off the
                # critical path
                g.memset(wu_idx[:], 0).then_inc(wu_sem, 1)
                g.wait_ge(wu_sem, 1)
                g.dma_gather(
                    wu_out[:].rearrange("p (b e) -> p b e", e=EMBED),
                    w_d.ap(),
                    wu_idx[:],
                    16,
                    g.to_reg(16),
                    EMBED,
                ).then_inc(wu_dma_sem, 16)
            g.wait_ge(ids_sem, 16)
            for c, (st, ch) in enumerate(zip(starts, chunk_sizes)):
                if scatter and ids_split and c == 1:
                    # chunk 1's idx columns ride the ACT-side DMA
                    g.wait_ge(sidx_sem, 16)
                g.dma_gather(
                    gath_t[:, st : st + ch].rearrange("p (b e) -> p b e", e=EMBED),
                    w_d.ap(),
                    idx_t[:, st // 16 : (st + ch) // 16],
                    ch,           # num_idxs
                    regs[ch],     # num_idxs_reg (all indices valid)
                    EMBED,        # elem_size (one table row)
                ).then_inc(gath_sems[c], 16)
            if scatter:
                # pre-generate the store descriptors on queue 1 while the
                # gathers transfer; src data is only read at trigger time
                for c, (st, ch) in enumerate(zip(starts, chunk_sizes)):
                    g.dma_scatter_add(
                        out_d.ap(),
                        gath_t[:, st : st + ch].rearrange(
                            "p (b e) -> p b e", e=EMBED
                        ),
                        idx_t[:, IDXW + st // 16 : IDXW + (st + ch) // 16],
                        ch,
                        regs[ch],
                        EMBED,
                        elem_step=EMBED,
                        prepare_only=True,
                        sem=sc_sems[c],
                        queue_num=1,
                    ).then_inc(prep_sem, 1)
                for c in range(chunks):
                    g.wait_ge(prep_sem, c + 1)
                    g.wait_ge(gath_sems[c], 16)
                    g.trigger_dma(1, queue_num=1)

        @block.sync
        def _(sp):
            # SP carries only what gates the first gather; with ids_split the
            # rest (chunk1 idx cols + scatter constants) rides ACT in parallel
            sp_cols = slice(0, chunk_sizes[0] // 16) if (scatter and ids_split) \
                else slice(0, ids_cols)
            if ids_drain_handoff:
                # drain waits for the HWDGE FIFO (data landed), then a cheap
                # engine sem-inc signals Pool -- skips the 900ns DMA sem prop
                sp.dma_start(idx_t[:, sp_cols], ids_d.ap()[:, sp_cols]).then_inc(
                    ids_dma_sem, 16
                )
                sp.drain().then_inc(ids_sem, 16)
            else:
                sp.dma_start(idx_t[:, sp_cols], ids_d.ap()[:, sp_cols]).then_inc(
                    ids_sem, 16
                )
            if scatter:
                return
            for c, (st, ch) in enumerate(zip(starts, chunk_sizes)):
                if split_store and c % 2 == 1:
                    continue  # handled by ACT below
                sp.wait_ge(gath_sems[c], 16)
                sp.dma_start(
                    out_v[:, st : st + ch], gath_t[:, st : st + ch]
                ).then_inc(st_sem, 16)
            if not no_store_wait:
                sp.wait_ge(st_sem, 16 * chunks)

        if scatter and ids_split:
            a_cols = slice(chunk_sizes[0] // 16, ids_cols)

            @block.scalar
            def _(act):
                act.dma_start(
                    idx_t[:, a_cols], ids_d.ap()[:, a_cols]
                ).then_inc(act_dma_sem, 16)
                act.drain().then_inc(sidx_sem, 16)

        if split_store and not scatter:

            @block.scalar
            def _(act):
                for c, (st, ch) in enumerate(zip(starts, chunk_sizes)):
                    if c % 2 == 0:
                        continue
                    act.wait_ge(gath_sems[c], 16)
                    act.dma_start(
                        out_v[:, st : st + ch], gath_t[:, st : st + ch]
                    ).then_inc(st_sem, 16)

    if strip_const_memsets:
        # The framework preamble memsets four const-* SBUF tiles this kernel
        # never reads; dropping them shortens the Pool preamble before the
        # entry barrier.
        import concourse.mybir as mybir

        blk = nc.m.functions[0].blocks[0]
        keep = [
            i
            for i in blk.instructions
            if not (
                isinstance(i, mybir.InstMemset)
                and i.outs
                and str(getattr(i.outs[0], "memref", "")).startswith("const-")
            )
        ]
        blk.instructions = keep

    if strip_entry_barrier:
        # The entry all-engine barrier only orders engine starts; all
        # cross-engine ordering in this kernel is carried by explicit
        # semaphores (which NRT resets between executions), so the ~600ns
        # barrier ahead of the ids DMA is dead weight.  The exit barrier
        # (completion guarantee) is kept.
        import concourse.mybir as mybir

        blk = nc.m.functions[0].blocks[0]
        blk.instructions = [
            i
            for i in blk.instructions
            if not isinstance(i, (mybir.InstDrain, mybir.InstEventSemaphore))
        ]

    nc.compile()
    return nc


def _get_nc(store_mode="hwdge"):
    if store_mode not in _NC_CACHE:
        _NC_CACHE[store_mode] = build_nc(store_mode=store_mode)
    return _NC_CACHE[store_mode]


def _wrap16(vals):
    """[NPC] -> [128, NPC//16] int16 in dma_gather's wrapped idx layout:
    value j at partition j%16, column j//16, replicated to all 8 gpsimd
    cores (16 partitions each)."""
    w = vals.reshape(IDXW, 16).T                         # [16, 64]
    return np.tile(w, (8, 1)).astype(np.int16)           # [128, 64]


def prep_ids(ids_flat, store_mode="hwdge"):
    """Per-core wrapped int16 idx arrays.

    hwdge:   gather position j looks up token t(j) = (j%128)*8 + j//128 so
             the SBUF tile stores contiguously (permuted layout).
    scatter: gather position j looks up token j (natural order); a constant
             identity block (scatter row indices) is appended so the
             dma_scatter_add writes out[j].
    """
    per_core = []
    ident = _wrap16(np.arange(NPC, dtype=np.int64))
    for c in range(N_CORES):
        shard = ids_flat[c * NPC : (c + 1) * NPC]
        if store_mode == "scatter":
            full = np.concatenate([_wrap16(shard), ident], axis=1)  # [128, 128]
        else:
            pos = shard.reshape(128, BLK).T.reshape(-1)  # pos[j] = shard[t(j)]
            full = _wrap16(pos)                          # [128, 64]
        per_core.append(np.ascontiguousarray(full))
    return per_core


DEFAULT_STORE_MODE = "scatter"


def run_spmd(inputs, trace=False, nc=None, store_mode=None):
    """Returns (output [4,2048,128] f32, BassKernelResults)."""
    from concourse.bass_utils import run_bass_kernel_spmd

    if store_mode is None:
        store_mode = DEFAULT_STORE_MODE
    ids = np.asarray(inputs["input_ids"]).reshape(-1).astype(np.int64)
    w = np.ascontiguousarray(np.asarray(inputs["weight"], dtype=np.float32))
    assert ids.shape == (N,) and w.shape == (VOCAB, EMBED)

    in_maps = [
        {"ids": ids_c, "weight": w} for ids_c in prep_ids(ids, store_mode)
    ]
    res = run_bass_kernel_spmd(
        nc if nc is not None else _get_nc(store_mode),
        in_maps,
        core_ids=list(range(N_CORES)),
        trace=trace,
    )
    shards = [r["out"] for r in res.results]
    out = np.concatenate(shards, axis=0).reshape(B, S, EMBED)
    return np.ascontiguousarray(out.astype(np.float32)), res


def kernel(**inputs):
    out, _ = run_spmd(inputs, trace=False)
    return out


# revision 54
# speedup vs baseline: 1.0447x; 1.0084x over previous
"""Embedding lookup (weight[input_ids]) on 8 Trainium2 NeuronCores.

Strategy: data-parallel over tokens. The 4x2048=8192 token ids are split
into 8 shards of 1024 tokens; every core holds the full [32000, 128] f32
table in HBM and uses the SWDGE dma_gather instruction to pull its 1024
rows (512 B each) directly from HBM into SBUF, then stores the gathered
block to its output shard with fully-contiguous DMAs.

Token->SBUF placement is chosen on the host so the SBUF->HBM store is
contiguous: gather position j handles token t = (j%128)*8 + j//128, which
lands token t's row at SBUF [partition t//8, block t%8].  Partition p then
holds tokens p*8..p*8+7 back to back, so the store AP is a plain
[128, 1024]f32 -> flat DRAM copy and the output shard comes out in natural
token order.

Pipeline (per core), store_mode="scatter" (default):
  SP  : ids DMA (HWDGE: wrapped gather indices + constant identity scatter
        indices in one [128,128]i16 tensor) -> drain -> engine-sem handoff
        to Pool (skips the ~900ns DMA-sem propagation)
  Pool: gpsimd 'mlp' ucode library load (overlaps the ids DMA), then on
        SWDGE queue 0 the gather split into (640, 384) chunks; while those
        transfer, the stores are pre-generated on SWDGE queue 1 as
        dma_scatter_add ops (identity indices onto the zero-initialized
        output - both runtime paths pre-zero/donate-zero ExternalOutputs)
        with prepare_only=True, then fired by trigger_dma the moment each
        gather's completion semaphore lands.  This removes the HWDGE
        store path entirely (no 625ns descriptor-gen + 650ns DGE-DMA delay
        per store) and keeps the SDMA engines almost continuously busy.
Completion is guaranteed by the block-exit engine drains rather than a
final sem wait.  The framework preamble is trimmed: the four const-memsets
(nothing reads them) and the entry all-engine barrier (all cross-engine
ordering here is carried by explicit semaphores, which the runtime resets
between executions) are stripped; the exit barrier is kept as the
completion guarantee.  The ids load is split: SP carries only the index
columns gating the first gather, ACT carries the rest in parallel.

TimelineSim (production cost model) estimate: ~6.9us per core; ~67% of
that is the serial SWDGE descriptor-generation chain (994ns fixed per op
x 4 ops: two gathers + two scatter preps, all on the Pool Q7), the rest
is ids-load latency and DMA transfers at modeled full bandwidth.
"""

import numpy as np

VOCAB = 32000
EMBED = 128
N_CORES = 8
B, S = 4, 2048
N = B * S                 # 8192 tokens total
NPC = N // N_CORES        # 1024 tokens per core
BLK = NPC // 128          # 8 blocks of 128 gather positions
IDXW = NPC // 16          # 64 idx columns in the wrapped idx layout

DEFAULT_CHUNKS = (640, 384)

_NC_CACHE = {}


def build_nc(chunk_sizes=DEFAULT_CHUNKS, split_store=False, no_gpsimd_drain=False,
             ids_drain_handoff=True, no_store_wait=True,
             strip_const_memsets=True, warmup_gather=False,
             store_mode=None, ids_split=True, strip_entry_barrier=True):
    """Build the per-core Bass program (identical on all 8 cores).

    store_mode:
      "hwdge"   - per-chunk SBUF->HBM DMACopy stores on SP (HWDGE)
      "scatter" - per-chunk dma_scatter_add (identity indices) onto the
                  zero-donated output, pre-generated on SWDGE queue 1 with
                  prepare_only and fired by trigger_dma as soon as the
                  matching gather's semaphore lands.  Skips the HWDGE
                  descriptor-gen + DGE-DMA-delay chain on the store path.
    """
    from contextlib import ExitStack

    import concourse.bacc as bacc
    import concourse.mybir as mybir
    from concourse import library_config

    if store_mode is None:
        store_mode = DEFAULT_STORE_MODE
    chunk_sizes = tuple(chunk_sizes)
    assert sum(chunk_sizes) == NPC
    assert all(c % 128 == 0 for c in chunk_sizes)
    chunks = len(chunk_sizes)
    starts = [sum(chunk_sizes[:i]) for i in range(chunks)]
    scatter = store_mode == "scatter"

    nc = bacc.Bacc("TRN2", target_bir_lowering=False, num_devices=N_CORES,
                   num_swdge_queues=2 if scatter else 1)

    # scatter mode: gather idx wrap (IDXW cols) + one 8-col identity wrap
    # (128 wide-unit indices) per chunk
    ids_cols = IDXW + 8 * chunks if scatter else IDXW
    ids_d = nc.dram_tensor("ids", [128, ids_cols], mybir.dt.int16,
                           kind="ExternalInput")
    w_d = nc.dram_tensor(
        "weight", [VOCAB, EMBED], mybir.dt.float32, kind="ExternalInput"
    )
    out_d = nc.dram_tensor(
        "out", [NPC, EMBED], mybir.dt.float32, kind="ExternalOutput"
    )

    with ExitStack() as stack:
        block = stack.enter_context(nc.Block(no_gpsimd_drain=no_gpsimd_drain))
        ids_sem = stack.enter_context(nc.semaphore("ids_sem"))
        ids_dma_sem = stack.enter_context(nc.semaphore("ids_dma_sem"))
        st_sem = stack.enter_context(nc.semaphore("st_sem"))
        gath_sems = [
            stack.enter_context(nc.semaphore(f"gath_sem{c}")) for c in range(chunks)
        ]
        if scatter:
            prep_sem = stack.enter_context(nc.semaphore("prep_sem"))
            sc_sems = [
                stack.enter_context(nc.semaphore(f"sc_sem{c}"))
                for c in range(chunks)
            ]
            ids_split = ids_split and chunks >= 2
            if ids_split:
                sidx_sem = stack.enter_context(nc.semaphore("sidx_sem"))
                act_dma_sem = stack.enter_context(nc.semaphore("act_dma_sem"))
        else:
            ids_split = False
        idx_t = stack.enter_context(
            nc.sbuf_tensor("idx_t", [128, ids_cols], mybir.dt.int16)
        )
        gath_t = stack.enter_context(
            nc.sbuf_tensor("gath_t", [128, NPC], mybir.dt.float32)
        )
        if warmup_gather:
            wu_sem = stack.enter_context(nc.semaphore("wu_sem"))
            wu_dma_sem = stack.enter_context(nc.semaphore("wu_dma_sem"))
            wu_idx = stack.enter_context(
                nc.sbuf_tensor("wu_idx", [128, 1], mybir.dt.int16)
            )
            wu_out = stack.enter_context(
                nc.sbuf_tensor("wu_out", [128, EMBED], mybir.dt.float32)
            )

        out_v = out_d.ap().rearrange("(p x) e -> p (x e)", p=128)  # [128, NPC]

        @block.gpsimd
        def _(g):
            g.load_library(library_config.mlp)
            # hoist the num_idxs registers so the ids wait attaches to the
            # first gather, not a register move
            regs = {}
            for ch in sorted(set(chunk_sizes)):
                regs[ch] = g.to_reg(ch)
            if warmup_gather:
                # run the gather ucode path once (row 0, 16 idxs) while the
                # ids DMA is in flight -- warms the Q7 icache off the
                # critical path
                g.memset(wu_idx[:], 0).then_inc(wu_sem, 1)
                g.wait_ge(wu_sem, 1)
                g.dma_gather(
                    wu_out[:].rearrange("p (b e) -> p b e", e=EMBED),
                    w_d.ap(),
                    wu_idx[:],
                    16,
                    g.to_reg(16),
                    EMBED,
                ).then_inc(wu_dma_sem, 16)
            g.wait_ge(ids_sem, 16)
            for c, (st, ch) in enumerate(zip(starts, chunk_sizes)):
                if scatter and ids_split and c == 1:
                    # chunk 1's idx columns ride the ACT-side DMA
                    g.wait_ge(sidx_sem, 16)
                g.dma_gather(
                    gath_t[:, st : st + ch].rearrange("p (b e) -> p b e", e=EMBED),
                    w_d.ap(),
                    idx_t[:, st // 16 : (st + ch) // 16],
                    ch,           # num_idxs
                    regs[ch],     # num_idxs_reg (all indices valid)
                    EMBED,        # elem_size (one table row)
                ).then_inc(gath_sems[c], 16)
            if scatter:
                # pre-generate the store descriptors on queue 1 while the
                # gathers transfer; src data is only read at trigger time.
                # Wide units: each of the 128 scatter indices moves one
                # partition's whole chunk-run (ch elements), so the prep's
                # per-idx DGE cost is paid 128x/chunk instead of ch x.
                reg128 = g.to_reg(128)
                for c, (st, ch) in enumerate(zip(starts, chunk_sizes)):
                    out_slice = out_d.ap()[st : st + ch, :].rearrange(
                        "(r k) e -> r (k e)", r=128
                    )  # [128, ch] rows of ch contiguous f32, stride ch
                    g.dma_scatter_add(
                        out_slice,
                        gath_t[:, st : st + ch].rearrange(
                            "p (b e) -> p b e", e=ch
                        ),  # [128, 1, ch]
                        idx_t[:, IDXW + 8 * c : IDXW + 8 * (c + 1)],
                        128,
                        reg128,
                        ch,
                        elem_step=ch,
                        prepare_only=True,
                        sem=sc_sems[c],
                        queue_num=1,
                    ).then_inc(prep_sem, 1)
                for c in range(chunks):
                    g.wait_ge(prep_sem, c + 1)
                    g.wait_ge(gath_sems[c], 16)
                    g.trigger_dma(1, queue_num=1)

        @block.sync
        def _(sp):
            # SP carries only what gates the first gather; with ids_split the
            # rest (chunk1 idx cols + scatter constants) rides ACT in parallel
            sp_cols = slice(0, chunk_sizes[0] // 16) if (scatter and ids_split) \
                else slice(0, ids_cols)
            if ids_drain_handoff:
                # drain waits for the HWDGE FIFO (data landed), then a cheap
                # engine sem-inc signals Pool -- skips the 900ns DMA sem prop
                sp.dma_start(idx_t[:, sp_cols], ids_d.ap()[:, sp_cols]).then_inc(
                    ids_dma_sem, 16
                )
                sp.drain().then_inc(ids_sem, 16)
            else:
                sp.dma_start(idx_t[:, sp_cols], ids_d.ap()[:, sp_cols]).then_inc(
                    ids_sem, 16
                )
            if scatter:
                return
            for c, (st, ch) in enumerate(zip(starts, chunk_sizes)):
                if split_store and c % 2 == 1:
                    continue  # handled by ACT below
                sp.wait_ge(gath_sems[c], 16)
                sp.dma_start(
                    out_v[:, st : st + ch], gath_t[:, st : st + ch]
                ).then_inc(st_sem, 16)
            if not no_store_wait:
                sp.wait_ge(st_sem, 16 * chunks)

        if scatter and ids_split:
            a_cols = slice(chunk_sizes[0] // 16, ids_cols)

            @block.scalar
            def _(act):
                act.dma_start(
                    idx_t[:, a_cols], ids_d.ap()[:, a_cols]
                ).then_inc(act_dma_sem, 16)
                act.drain().then_inc(sidx_sem, 16)

        if split_store and not scatter:

            @block.scalar
            def _(act):
                for c, (st, ch) in enumerate(zip(starts, chunk_sizes)):
                    if c % 2 == 0:
                        continue
                    act.wait_ge(gath_sems[c], 16)
                    act.dma_start(
                        out_v[:, st : st + ch], gath_t[:, st : st + ch]
                    ).then_inc(st_sem, 16)

    if strip_const_memsets:
        # The framework preamble memsets four const-* SBUF tiles this kernel
        # never reads; dropping them shortens the Pool preamble before the
        # entry barrier.
        import concourse.mybir as mybir

        blk = nc.m.functions[0].blocks[0]
        keep = [
            i
            for i in blk.instructions
            if not (
                isinstance(i, mybir.InstMemset)
                and i.outs
                and str(getattr(i.outs[0], "memref", "")).startswith("const-")
            )
        ]
        blk.instructions = keep

    if strip_entry_barrier:
        # The entry all-engine barrier only orders engine starts; all
        # cross-engine ordering in this kernel is carried by explicit
        # semaphores (which NRT resets between executions), so the ~600ns
        # barrier ahead of the ids DMA is dead weight.  The exit barrier
        # (completion guarantee) is kept.
        import concourse.mybir as mybir

        blk = nc.m.functions[0].blocks[0]
        blk.instructions = [
            i
            for i in blk.instructions
            if not isinstance(i, (mybir.InstDrain, mybir.InstEventSemaphore))
        ]

    nc.compile()
    return nc


def _get_nc(store_mode="hwdge"):
    if store_mode not in _NC_CACHE:
        _NC_CACHE[store_mode] = build_nc(store_mode=store_mode)
    return _NC_CACHE[store_mode]


def _wrap16(vals):
    """[n] -> [128, n//16] int16 in the SWDGE wrapped idx layout: value j at
    partition j%16, column j//16, replicated to all 8 gpsimd cores (16
    partitions each)."""
    w = vals.reshape(-1, 16).T                           # [16, n/16]
    return np.tile(w, (8, 1)).astype(np.int16)           # [128, n/16]


def prep_ids(ids_flat, store_mode="hwdge", chunk_sizes=DEFAULT_CHUNKS):
    """Per-core wrapped int16 idx arrays.

    hwdge:   gather position j looks up token t(j) = (j%128)*8 + j//128 so
             the SBUF tile stores contiguously (permuted layout).
    scatter: per chunk (R = ch/128 blocks), gather position b*128+p looks up
             chunk token p*R+b, so partition p holds its R chunk rows
             back-to-back; appended per-chunk identity wraps (128 wide-unit
             indices each) drive the dma_scatter_add stores.
    """
    chunk_sizes = tuple(chunk_sizes)
    starts = [sum(chunk_sizes[:i]) for i in range(len(chunk_sizes))]
    ident128 = _wrap16(np.arange(128, dtype=np.int64))   # [128, 8]
    per_core = []
    for c in range(N_CORES):
        shard = ids_flat[c * NPC : (c + 1) * NPC]
        if store_mode == "scatter":
            gw = []
            for st, ch in zip(starts, chunk_sizes):
                sub = shard[st : st + ch]
                pos = sub.reshape(128, ch // 128).T.reshape(-1)
                gw.append(_wrap16(pos))
            full = np.concatenate(
                gw + [ident128] * len(chunk_sizes), axis=1
            )  # [128, IDXW + 8*chunks]
        else:
            pos = shard.reshape(128, BLK).T.reshape(-1)  # pos[j] = shard[t(j)]
            full = _wrap16(pos)                          # [128, 64]
        per_core.append(np.ascontiguousarray(full))
    return per_core


DEFAULT_STORE_MODE = "scatter"


def run_spmd(inputs, trace=False, nc=None, store_mode=None):
    """Returns (output [4,2048,128] f32, BassKernelResults)."""
    from concourse.bass_utils import run_bass_kernel_spmd

    if store_mode is None:
        store_mode = DEFAULT_STORE_MODE
    ids = np.asarray(inputs["input_ids"]).reshape(-1).astype(np.int64)
    w = np.ascontiguousarray(np.asarray(inputs["weight"], dtype=np.float32))
    assert ids.shape == (N,) and w.shape == (VOCAB, EMBED)

    in_maps = [
        {"ids": ids_c, "weight": w} for ids_c in prep_ids(ids, store_mode)
    ]
    res = run_bass_kernel_spmd(
        nc if nc is not None else _get_nc(store_mode),
        in_maps,
        core_ids=list(range(N_CORES)),
        trace=trace,
    )
    shards = [r["out"] for r in res.results]
    out = np.concatenate(shards, axis=0).reshape(B, S, EMBED)
    return np.ascontiguousarray(out.astype(np.float32)), res


def kernel(**inputs):
    out, _ = run_spmd(inputs, trace=False)
    return out
